# revision 9
# baseline (speedup 1.0000x reference)
"""CRF log-likelihood on 8 TRN2 NeuronCores — time-parallel forward scan.

Strategy:
- Numerator (cheap gathers over (S,B)) computed on host (f64).
- Log-partition via the linear-space forward recurrence
      x_{t}[j,b] = g_t[j,b] * sum_i E[i,j] * x_{t-1}[i,b]
  with E = exp(transitions), g_t = exp(em_t - c_t), c_t a host-side
  per-step centering constant.
- Time-parallel decomposition: the per-step operator diag(g_t) E^T is a
  positive map whose Birkhoff (Hilbert-metric) contraction ratio is
  tanh(D/4) ~= 0.1 for transitions ~ U(-0.1, 0.1).  A chain started W
  steps early from the uniform vector recovers the true state DIRECTION
  to ~0.4 * 0.1^(W-1); per-segment scalar factors telescope:
      log Z_b = sum_p log r_p[b] - sum_{p != 0} log sigma_p[b] + sum_t c_t
  where sigma_p = colsum of the chain state at its segment-start boundary
  and r_p = colsum (endv-weighted for the last segment) at its end
  boundary.  Chain 0 starts from the exact alpha_0, so its sigma is not
  subtracted.
- 8*K chains total, K per core; every chain processes all 256 batch
  columns.  Per step: one 128x128 @ 128x256 bf16 matmul against the
  stationary E, then the elementwise multiply by g, column-split across
  engines: DVE multiplies CA columns straight out of PSUM; Act copies the
  remaining CB columns PSUM->SBUF (GPSIMD has no PSUM port) and GPSIMD
  multiplies those.
"""

import sys

import numpy as np

sys.path.insert(0, "/opt/trn_rl_repo")

S, B, T = 512, 256, 128
NCORES = 8

# ---- time-parallel configuration ------------------------------------------
K = 4                      # chains (time segments) per core
P = NCORES * K             # global chains
LSLOT = [16, 16, 16, 15]   # real steps per chain, by within-core slot
W0 = 7                     # slot-0 "warmup" (real work on chain 0)
WR = 4                     # warmup steps, other slots
WU = [W0] + [WR] * (K - 1)
NSTEPS = [WU[k] + LSLOT[k] for k in range(K)]      # steps per slot
MAXN = max(NSTEPS)
TOTROWS = sum(NSTEPS)
assert 7 * LSLOT[0] + NSTEPS[0] + 8 * sum(LSLOT[1:]) == S - 1

# execution-ordered (slot, step) pairs; G rows are stored in this order
ORDER = [(k, s) for s in range(MAXN) for k in range(K) if s < NSTEPS[k]]
RIDX = {ks: i for i, ks in enumerate(ORDER)}

CA = 192                   # columns multiplied by DVE directly from PSUM
CB = B - CA                # columns via Act copy -> GPSIMD multiply
DMA_CHUNK = 16             # G rows per DMA
WARMCOLS = 256             # width of each PE-warming dummy matmul
XBUFS = 4                  # ring depth for x / tmp tiles

_NC_CACHE = {}


def _chain_end(p):
    """Last real timestep (1-based g index) covered by global chain p."""
    e = NSTEPS[0]
    for q in range(1, p + 1):
        e += LSLOT[q // NCORES]
    return e


def _build_nc():
    import concourse.bass as bass
    import concourse.mybir as mybir
    import concourse.tile as tile
    from concourse import bacc

    f32 = mybir.dt.float32
    bf16 = mybir.dt.bfloat16
    nc = bacc.Bacc(None, target_bir_lowering=False)

    E_ext = nc.declare_dram_parameter("E", [T, T], bf16, isOutput=False)
    g_ext = nc.declare_dram_parameter("G", [T, TOTROWS, B], bf16, isOutput=False)
    x0_ext = nc.declare_dram_parameter("x0", [T, K * B], bf16, isOutput=False)
    rv_ext = nc.declare_dram_parameter("rv", [T, K + 1], bf16, isOutput=False)
    out_ext = nc.declare_dram_parameter("out", [1, 2 * K * B], f32, isOutput=True)

    with tile.TileContext(nc) as tc:
        with (
            tc.tile_pool(name="const", bufs=1) as constp,
            tc.tile_pool(name="gbuf", bufs=1) as gp,
            tc.tile_pool(name="xbuf", bufs=XBUFS) as xp,
            tc.tile_pool(name="tmp", bufs=XBUFS) as tp,
            tc.tile_pool(name="res", bufs=1) as resp,
            tc.tile_pool(name="psum", bufs=1, space=bass.MemorySpace.PSUM) as pp,
            tc.tile_pool(name="psum_sr", bufs=2, space=bass.MemorySpace.PSUM) as sp,
        ):
            E_t = constp.tile([T, T], bf16)
            rv_t = constp.tile([T, K + 1], bf16)
            # startup-critical inputs first: the Sync queue issues DMAs in
            # order, so x0/E/rv must not sit behind the big G stream
            nc.sync.dma_start(E_t[:], E_ext[:, :])
            nc.sync.dma_start(rv_t[:], rv_ext[:, :])

            x = []
            for k in range(K):
                xk = xp.tile([T, B], bf16, tag=f"x{k}")
                nc.sync.dma_start(xk[:], x0_ext[:, k * B:(k + 1) * B])
                x.append(xk)

            G_t = gp.tile([T, TOTROWS, B], bf16)
            bounds = [0, 2, 6, 16]
            while bounds[-1] < TOTROWS:
                bounds.append(min(bounds[-1] + DMA_CHUNK, TOTROWS))
            for r0, r1 in zip(bounds, bounds[1:]):
                nc.sync.dma_start(G_t[:, r0:r1, :], g_ext[:, r0:r1, :])

            res_t = resp.tile([1, 2 * K * B], f32)

            for s in range(MAXN):
                for k in range(K):
                    if s >= NSTEPS[k]:
                        continue
                    row = RIDX[(k, s)]
                    p = pp.tile([T, B], f32, tag=f"p{k}")
                    # ldweights-only warming: keeps the PE weight path
                    # streaming (HAM throttle avoidance) at ~1/4 the cost of
                    # a dummy matmul; loads E, same as the real matmul after
                    nc.tensor.ldweights(E_t[:])
                    nc.tensor.ldweights(E_t[:])
                    nc.tensor.matmul(p[:], E_t[:], x[k][:])
                    xn = xp.tile([T, B], bf16, tag=f"x{k}")
                    nc.vector.tensor_mul(xn[:, :CA], p[:, :CA], G_t[:, row, :CA])
                    tmp = tp.tile([T, CB], bf16, tag=f"t{k}")
                    nc.scalar.copy(tmp[:], p[:, CA:])
                    nc.gpsimd.tensor_mul(xn[:, CA:], tmp[:], G_t[:, row, CA:])
                    x[k] = xn
                    if s == WU[k] - 1:
                        sg = sp.tile([1, B], f32, tag="sr")
                        nc.tensor.matmul(sg[:], rv_t[:, 0:1], xn[:])
                        nc.scalar.copy(res_t[0:1, k * B:(k + 1) * B], sg[:])
                    if s == NSTEPS[k] - 1:
                        rr = sp.tile([1, B], f32, tag="sr")
                        nc.tensor.matmul(rr[:], rv_t[:, 1 + k:2 + k], xn[:])
                        nc.scalar.copy(
                            res_t[0:1, (K + k) * B:(K + k + 1) * B], rr[:])
                        # stream this r row out immediately (tail shaving)
                        nc.sync.dma_start(
                            out_ext[:, (K + k) * B:(K + k + 1) * B],
                            res_t[0:1, (K + k) * B:(K + k + 1) * B])

            nc.sync.dma_start(out_ext[:, :K * B], res_t[0:1, :K * B])

    nc.compile()
    return nc


def _numerator(emissions, tags, mask, start_transitions, end_transitions, transitions):
    maskf = mask.astype(np.float64)
    em_scores = np.take_along_axis(emissions, tags[:, :, None], axis=2)[..., 0]
    llh = start_transitions[tags[0]].astype(np.float64)
    llh = llh + np.sum(em_scores[:-1] * maskf[:-1], axis=0)
    llh = llh + np.sum(transitions[tags[:-1], tags[1:]] * maskf[1:], axis=0)
    last_idx = np.sum(mask.astype(np.int64), axis=0) - 1
    last_tags = np.take_along_axis(tags, last_idx[None, :], axis=0)[0]
    llh = llh + end_transitions[last_tags]
    llh = llh + em_scores[-1] * maskf[-1]
    return llh  # (B,) float64


def _logz_host_fallback(emissions, mask, start_transitions, end_transitions, transitions):
    # General-mask fallback (spec mask is all ones, so normally unused).
    lp = start_transitions[None, :] + emissions[0]
    lp = lp.astype(np.float64)
    tr = transitions.astype(np.float64)
    for t in range(1, emissions.shape[0]):
        sc = lp[:, :, None] + tr[None, :, :] + emissions[t][:, None, :].astype(np.float64)
        m = sc.max(axis=1, keepdims=True)
        new = np.log(np.exp(sc - m).sum(axis=1)) + m[:, 0, :]
        lp = np.where(mask[t][:, None] > 0, new, lp)
    sc = lp + end_transitions[None, :]
    m = sc.max(axis=1, keepdims=True)
    return np.log(np.exp(sc - m).sum(axis=1)) + m[:, 0]


def _host_inputs(emissions, start_transitions, end_transitions, transitions):
    """Build per-core device inputs. Returns (in_maps, c_sum)."""
    import ml_dtypes

    bf16 = ml_dtypes.bfloat16

    em64 = emissions.astype(np.float64)
    mx = em64.reshape(S, -1).max(axis=1)
    c = np.log(np.exp(em64 - mx[:, None, None]).reshape(S, -1).sum(axis=1)) + mx - np.log(B)

    E = np.exp(transitions).astype(bf16)  # (T,T), [i,j]
    # g for t=1..511 in [tag, t, batch] layout
    g_all = np.exp(em64[1:] - c[1:, None, None]).astype(np.float32)  # (S-1, B, T)
    g_all = np.ascontiguousarray(g_all.transpose(2, 0, 1)).astype(bf16)  # (T, S-1, B)

    x0_a = np.exp(
        start_transitions[:, None].astype(np.float64) + em64[0].T - c[0]
    ).astype(bf16)  # (T, B): exact alpha_0 for global chain 0

    in_maps = []
    for cix in range(NCORES):
        tsteps = np.empty(TOTROWS, dtype=np.int64)
        for (k, s), ridx in RIDX.items():
            p = k * NCORES + cix
            tsteps[ridx] = _chain_end(p) - NSTEPS[k] + s  # 0-based into g_all
        G_core = np.ascontiguousarray(g_all[:, tsteps, :])

        x0 = np.ones((T, K * B), dtype=bf16)
        if cix == 0:
            x0[:, 0:B] = x0_a

        rv = np.ones((T, K + 1), dtype=bf16)
        if cix == NCORES - 1:
            rv[:, K] = np.exp(end_transitions).astype(bf16)

        in_maps.append({"E": E, "G": G_core, "x0": x0, "rv": rv})
    return in_maps, float(c.sum())


def _assemble(outs, c_sum):
    """outs: list of (1, 2*K*B) f32 per core -> log_z (B,) f64."""
    log_z = np.full(B, c_sum, dtype=np.float64)
    for cix in range(NCORES):
        o = np.asarray(outs[cix], dtype=np.float64).reshape(2 * K, B)
        for k in range(K):
            log_z += np.log(o[K + k])          # r
            if not (cix == 0 and k == 0):
                log_z -= np.log(o[k])          # sigma
    return log_z


PROFILE = False
LAST_RESULT = None


def kernel(emissions, tags, mask, start_transitions, end_transitions, transitions):
    global LAST_RESULT
    emissions = np.asarray(emissions, dtype=np.float32)
    tags = np.asarray(tags, dtype=np.int32)
    mask = np.asarray(mask, dtype=np.int32)
    start_transitions = np.asarray(start_transitions, dtype=np.float32)
    end_transitions = np.asarray(end_transitions, dtype=np.float32)
    transitions = np.asarray(transitions, dtype=np.float32)

    llh = _numerator(emissions, tags, mask, start_transitions, end_transitions, transitions)

    if not np.all(mask == 1):
        log_z = _logz_host_fallback(
            emissions, mask, start_transitions, end_transitions, transitions
        )
        return np.asarray(np.sum(llh - log_z), dtype=np.float32)

    in_maps, c_sum = _host_inputs(
        emissions, start_transitions, end_transitions, transitions
    )

    from concourse.bass_utils import run_bass_kernel_spmd

    if "nc" not in _NC_CACHE:
        _NC_CACHE["nc"] = _build_nc()
    nc = _NC_CACHE["nc"]

    r = run_bass_kernel_spmd(
        nc, in_maps, core_ids=list(range(NCORES)), trace=PROFILE
    )
    LAST_RESULT = r
    outs = [r.results[cix]["out"] for cix in range(NCORES)]
    log_z = _assemble(outs, c_sum)

    return np.asarray(np.sum(llh - log_z), dtype=np.float32)


if __name__ == "__main__":
    rng = np.random.default_rng(0)
    ins = {
        "emissions": rng.standard_normal((S, B, T), dtype=np.float32),
        "tags": rng.integers(0, T, (S, B)).astype(np.int32),
        "mask": np.ones((S, B), np.int32),
        "start_transitions": rng.uniform(-0.1, 0.1, (T,)).astype(np.float32),
        "end_transitions": rng.uniform(-0.1, 0.1, (T,)).astype(np.float32),
        "transitions": rng.uniform(-0.1, 0.1, (T, T)).astype(np.float32),
    }
    print(kernel(**ins))


# revision 11
# speedup vs baseline: 1.0857x; 1.0857x over previous
"""CRF log-likelihood on 8 TRN2 NeuronCores — time-parallel forward scan.

Strategy:
- Numerator (cheap gathers over (S,B)) computed on host (f64).
- Log-partition via the linear-space forward recurrence
      x_{t}[j,b] = g_t[j,b] * sum_i E[i,j] * x_{t-1}[i,b]
  with E = exp(transitions), g_t = exp(em_t - c_t), c_t a host-side
  per-step centering constant.
- Time-parallel decomposition: the per-step operator diag(g_t) E^T is a
  positive map whose Birkhoff (Hilbert-metric) contraction ratio is
  tanh(D/4) ~= 0.1 for transitions ~ U(-0.1, 0.1).  A chain started W
  steps early from the uniform vector recovers the true state DIRECTION
  to ~0.4 * 0.1^(W-1); per-segment scalar factors telescope:
      log Z_b = sum_p log r_p[b] - sum_{p != 0} log sigma_p[b] + sum_t c_t
  where sigma_p = colsum of the chain state at its segment-start boundary
  and r_p = colsum (endv-weighted for the last segment) at its end
  boundary.  Chain 0 starts from the exact alpha_0, so its sigma is not
  subtracted.
- 8*K chains total, K per core; every chain processes all 256 batch
  columns.  Per step: one 128x128 @ 128x256 bf16 matmul against the
  stationary E, then the elementwise multiply by g, column-split across
  engines: DVE multiplies CA columns straight out of PSUM; Act copies the
  remaining CB columns PSUM->SBUF (GPSIMD has no PSUM port) and GPSIMD
  multiplies those.
"""

import sys

import numpy as np

sys.path.insert(0, "/opt/trn_rl_repo")

S, B, T = 512, 256, 128
NCORES = 8

# ---- time-parallel configuration ------------------------------------------
K = 4                      # chains (time segments) per core
P = NCORES * K             # global chains
LSLOT = [16, 16, 16, 15]   # real steps per chain, by within-core slot
W0 = 7                     # slot-0 "warmup" (real work on chain 0)
WR = 4                     # warmup steps, other slots
WU = [W0] + [WR] * (K - 1)
NSTEPS = [WU[k] + LSLOT[k] for k in range(K)]      # steps per slot
MAXN = max(NSTEPS)
TOTROWS = sum(NSTEPS)
assert 7 * LSLOT[0] + NSTEPS[0] + 8 * sum(LSLOT[1:]) == S - 1

# execution-ordered (slot, step) pairs; G rows are stored in this order
ORDER = [(k, s) for s in range(MAXN) for k in range(K) if s < NSTEPS[k]]
RIDX = {ks: i for i, ks in enumerate(ORDER)}

CA = 192                   # columns multiplied by DVE directly from PSUM
CB = B - CA                # columns via Act copy -> GPSIMD multiply
DMA_CHUNK = 16             # G rows per DMA
WARMCOLS = 256             # width of each PE-warming dummy matmul
XBUFS = 4                  # ring depth for x / tmp tiles

_NC_CACHE = {}


def _chain_end(p):
    """Last real timestep (1-based g index) covered by global chain p."""
    e = NSTEPS[0]
    for q in range(1, p + 1):
        e += LSLOT[q // NCORES]
    return e


def _build_nc():
    import concourse.bass as bass
    import concourse.mybir as mybir
    import concourse.tile as tile
    from concourse import bacc

    f32 = mybir.dt.float32
    bf16 = mybir.dt.bfloat16
    nc = bacc.Bacc(None, target_bir_lowering=False)

    cst_ext = nc.declare_dram_parameter(
        "cst", [T, K * B + T + (K + 1)], bf16, isOutput=False)
    g_ext = nc.declare_dram_parameter("G", [T, TOTROWS, B], bf16, isOutput=False)
    out_ext = nc.declare_dram_parameter("out", [1, 2 * K * B], f32, isOutput=True)

    with tile.TileContext(nc) as tc:
        with (
            tc.tile_pool(name="const", bufs=1) as constp,
            tc.tile_pool(name="gbuf", bufs=1) as gp,
            tc.tile_pool(name="xbuf", bufs=XBUFS) as xp,
            tc.tile_pool(name="tmp", bufs=XBUFS) as tp,
            tc.tile_pool(name="res", bufs=1) as resp,
            tc.tile_pool(name="psum", bufs=1, space=bass.MemorySpace.PSUM) as pp,
            tc.tile_pool(name="psum_sr", bufs=2, space=bass.MemorySpace.PSUM) as sp,
        ):
            # one combined DMA for all startup-critical constants:
            # [x0 (K*B) | E (T) | rv (K+1)] columns
            cst = constp.tile([T, K * B + T + (K + 1)], bf16)
            nc.sync.dma_start(cst[:], cst_ext[:, :])
            E_t = cst[:, CST_E:CST_E + T]
            rv_t = cst[:, CST_RV:CST_RV + K + 1]

            G_t = gp.tile([T, TOTROWS, B], bf16)
            assert G_BOUNDS[-1] == TOTROWS
            for r0, r1 in zip(G_BOUNDS, G_BOUNDS[1:]):
                nc.sync.dma_start(G_t[:, r0:r1, :], g_ext[:, r0:r1, :])

            res_t = resp.tile([1, 2 * K * B], f32)

            # startup warm-up burst: back-to-back matmuls ramp the PE to
            # full clock (HAM) while the G stream is still in flight
            warm = sp.tile([T, T], f32, tag="warm", bufs=1)
            for _ in range(NBURST):
                nc.tensor.matmul(warm[:], E_t, E_t)

            x = [cst[:, k * B:(k + 1) * B] for k in range(K)]

            for s in range(MAXN):
                for k in range(K):
                    if s >= NSTEPS[k]:
                        continue
                    row = RIDX[(k, s)]
                    p = pp.tile([T, B], f32, tag=f"p{k}")
                    nc.tensor.matmul(p[:], E_t, x[k][:, :])
                    xn = xp.tile([T, B], bf16, tag=f"x{k}")
                    nc.vector.tensor_mul(xn[:, :CA], p[:, :CA], G_t[:, row, :CA])
                    tmp = tp.tile([T, CB], bf16, tag=f"t{k}")
                    nc.scalar.copy(tmp[:], p[:, CA:])
                    nc.gpsimd.tensor_mul(xn[:, CA:], tmp[:], G_t[:, row, CA:])
                    x[k] = xn
                    if s == WU[k] - 1:
                        sg = sp.tile([1, B], f32, tag="sr")
                        nc.tensor.matmul(sg[:], rv_t[:, 0:1], xn[:])
                        nc.scalar.copy(res_t[0:1, k * B:(k + 1) * B], sg[:])
                    if s == NSTEPS[k] - 1:
                        rr = sp.tile([1, B], f32, tag="sr")
                        nc.tensor.matmul(rr[:], rv_t[:, 1 + k:2 + k], xn[:])
                        nc.scalar.copy(
                            res_t[0:1, (K + k) * B:(K + k + 1) * B], rr[:])
                        # stream this r row out immediately (tail shaving)
                        nc.sync.dma_start(
                            out_ext[:, (K + k) * B:(K + k + 1) * B],
                            res_t[0:1, (K + k) * B:(K + k + 1) * B])

            nc.sync.dma_start(out_ext[:, :K * B], res_t[0:1, :K * B])

    nc.compile()
    return nc


def _numerator(emissions, tags, mask, start_transitions, end_transitions, transitions):
    maskf = mask.astype(np.float64)
    em_scores = np.take_along_axis(emissions, tags[:, :, None], axis=2)[..., 0]
    llh = start_transitions[tags[0]].astype(np.float64)
    llh = llh + np.sum(em_scores[:-1] * maskf[:-1], axis=0)
    llh = llh + np.sum(transitions[tags[:-1], tags[1:]] * maskf[1:], axis=0)
    last_idx = np.sum(mask.astype(np.int64), axis=0) - 1
    last_tags = np.take_along_axis(tags, last_idx[None, :], axis=0)[0]
    llh = llh + end_transitions[last_tags]
    llh = llh + em_scores[-1] * maskf[-1]
    return llh  # (B,) float64


def _logz_host_fallback(emissions, mask, start_transitions, end_transitions, transitions):
    # General-mask fallback (spec mask is all ones, so normally unused).
    lp = start_transitions[None, :] + emissions[0]
    lp = lp.astype(np.float64)
    tr = transitions.astype(np.float64)
    for t in range(1, emissions.shape[0]):
        sc = lp[:, :, None] + tr[None, :, :] + emissions[t][:, None, :].astype(np.float64)
        m = sc.max(axis=1, keepdims=True)
        new = np.log(np.exp(sc - m).sum(axis=1)) + m[:, 0, :]
        lp = np.where(mask[t][:, None] > 0, new, lp)
    sc = lp + end_transitions[None, :]
    m = sc.max(axis=1, keepdims=True)
    return np.log(np.exp(sc - m).sum(axis=1)) + m[:, 0]


def _host_inputs(emissions, start_transitions, end_transitions, transitions):
    """Build per-core device inputs. Returns (in_maps, c_sum)."""
    import ml_dtypes

    bf16 = ml_dtypes.bfloat16

    em64 = emissions.astype(np.float64)
    mx = em64.reshape(S, -1).max(axis=1)
    c = np.log(np.exp(em64 - mx[:, None, None]).reshape(S, -1).sum(axis=1)) + mx - np.log(B)

    E = np.exp(transitions).astype(bf16)  # (T,T), [i,j]
    # g for t=1..511 in [tag, t, batch] layout
    g_all = np.exp(em64[1:] - c[1:, None, None]).astype(np.float32)  # (S-1, B, T)
    g_all = np.ascontiguousarray(g_all.transpose(2, 0, 1)).astype(bf16)  # (T, S-1, B)

    x0_a = np.exp(
        start_transitions[:, None].astype(np.float64) + em64[0].T - c[0]
    ).astype(bf16)  # (T, B): exact alpha_0 for global chain 0

    in_maps = []
    for cix in range(NCORES):
        tsteps = np.empty(TOTROWS, dtype=np.int64)
        for (k, s), ridx in RIDX.items():
            p = k * NCORES + cix
            tsteps[ridx] = _chain_end(p) - NSTEPS[k] + s  # 0-based into g_all
        G_core = np.ascontiguousarray(g_all[:, tsteps, :])

        cst = np.ones((T, K * B + T + (K + 1)), dtype=bf16)
        if cix == 0:
            cst[:, 0:B] = x0_a
        cst[:, CST_E:CST_E + T] = E
        if cix == NCORES - 1:
            cst[:, CST_RV + K] = np.exp(end_transitions).astype(bf16)

        in_maps.append({"cst": cst, "G": G_core})
    return in_maps, float(c.sum())


def _assemble(outs, c_sum):
    """outs: list of (1, 2*K*B) f32 per core -> log_z (B,) f64."""
    log_z = np.full(B, c_sum, dtype=np.float64)
    for cix in range(NCORES):
        o = np.asarray(outs[cix], dtype=np.float64).reshape(2 * K, B)
        for k in range(K):
            log_z += np.log(o[K + k])          # r
            if not (cix == 0 and k == 0):
                log_z -= np.log(o[k])          # sigma
    return log_z


PROFILE = False
LAST_RESULT = None


def kernel(emissions, tags, mask, start_transitions, end_transitions, transitions):
    global LAST_RESULT
    emissions = np.asarray(emissions, dtype=np.float32)
    tags = np.asarray(tags, dtype=np.int32)
    mask = np.asarray(mask, dtype=np.int32)
    start_transitions = np.asarray(start_transitions, dtype=np.float32)
    end_transitions = np.asarray(end_transitions, dtype=np.float32)
    transitions = np.asarray(transitions, dtype=np.float32)

    llh = _numerator(emissions, tags, mask, start_transitions, end_transitions, transitions)

    if not np.all(mask == 1):
        log_z = _logz_host_fallback(
            emissions, mask, start_transitions, end_transitions, transitions
        )
        return np.asarray(np.sum(llh - log_z), dtype=np.float32)

    in_maps, c_sum = _host_inputs(
        emissions, start_transitions, end_transitions, transitions
    )

    from concourse.bass_utils import run_bass_kernel_spmd

    if "nc" not in _NC_CACHE:
        _NC_CACHE["nc"] = _build_nc()
    nc = _NC_CACHE["nc"]

    r = run_bass_kernel_spmd(
        nc, in_maps, core_ids=list(range(NCORES)), trace=PROFILE
    )
    LAST_RESULT = r
    outs = [r.results[cix]["out"] for cix in range(NCORES)]
    log_z = _assemble(outs, c_sum)

    return np.asarray(np.sum(llh - log_z), dtype=np.float32)


if __name__ == "__main__":
    rng = np.random.default_rng(0)
    ins = {
        "emissions": rng.standard_normal((S, B, T), dtype=np.float32),
        "tags": rng.integers(0, T, (S, B)).astype(np.int32),
        "mask": np.ones((S, B), np.int32),
        "start_transitions": rng.uniform(-0.1, 0.1, (T,)).astype(np.float32),
        "end_transitions": rng.uniform(-0.1, 0.1, (T,)).astype(np.float32),
        "transitions": rng.uniform(-0.1, 0.1, (T, T)).astype(np.float32),
    }
    print(kernel(**ins))
